# revision 1
# baseline (speedup 1.0000x reference)
"""Bass/Trainium2 kernel for nn_EuclideanGraphEncoder (GCN message passing).

Strategy: data-parallel over the batch (4 graphs per core, 8 cores),
weights replicated, no collectives. The adjacency matrix is transposed
and downcast to fp16 on the host during sharding so the aggregation
matmul (adj @ msg) can contract over SBUF partitions directly; per-layer
biases enter the aggregation PSUM as rank-1 matmuls against host-computed
exact row-sums of adj.

Device-side layout: h is kept transposed [hid=128 partitions, n=1024]
in SBUF fp16. Per layer:
  msg[n,k]  = h @ Wl       (8x K=128/M=128/N=128 matmuls, PSUM->SBUF fp16)
  aggT[k,n] = msg.T @ adjT (2 n-tiles x 8 accumulating K=128/N=512 matmuls
                            + 1 rank-1 bias matmul each)
  hT        = relu(aggT)   (ACT, PSUM->SBUF fp16 cast)
Projection returns to natural [n, 64] layout with a rank-1 bias matmul,
the node mask applied as a per-partition ACT scale, output fp32.
"""

import sys
from contextlib import ExitStack

import numpy as np

try:
    import concourse.bass as bass
except ImportError:  # fall back to the repo checkout
    sys.path.insert(0, "/opt/trn_rl_repo")
    import concourse.bass as bass

import concourse.tile as tile
from concourse import bacc, mybir
from concourse.bass_utils import run_bass_kernel_spmd

B, N, IN_DIM, HID, OUT = 32, 1024, 64, 128, 64
NUM_LAYERS = 3
N_CORES = 8
BPC = B // N_CORES  # graphs per core
NT = N // 512  # aggregation free-dim tiles
NC8 = N // 128  # node chunks of 128

FP16 = mybir.dt.float16
FP32 = mybir.dt.float32
RELU = mybir.ActivationFunctionType.Relu
COPY = mybir.ActivationFunctionType.Copy

# Per-layer power-of-2 scales: SBUF h/msg tiles hold h_true / S[i] so fp16
# never overflows (true agg magnitudes reach ~5e6). Scale hops are exact
# (powers of two) and ride existing instructions: the relu ACT scale, the
# host-prescaled bias operands, and the final mask scale (fp32).
S = [1.0, 64.0, 16384.0, 4194304.0]
ONES_VAL = 2.0 ** -11  # proj bias rank-1: ones * (b_proj * 2^11 / S[3])


def _kernel_body(ctx, tc, out, adjT, xT, maskT, w_embed, wl, blT, w_proj, b_proj):
    nc = tc.nc

    # All-resident pools (bufs = total tiles): load slots are write-once, so
    # load DMAs carry only their queue-ordering wait. adj is loaded as
    # per-chunk tiles so each aggregation matmul releases as soon as its own
    # 256KB slab lands instead of waiting for a whole graph (or all graphs).
    consts = ctx.enter_context(tc.tile_pool(name="consts", bufs=1))
    adj_pool = ctx.enter_context(tc.tile_pool(name="adj", bufs=BPC * NC8))
    xt_pool = ctx.enter_context(tc.tile_pool(name="xt", bufs=BPC))
    mask_pool = ctx.enter_context(tc.tile_pool(name="mask", bufs=BPC))
    h_pool = ctx.enter_context(tc.tile_pool(name="h", bufs=BPC + 3))
    msg_pool = ctx.enter_context(tc.tile_pool(name="msg", bufs=BPC + 2))
    o_pool = ctx.enter_context(tc.tile_pool(name="o", bufs=2))
    psA = ctx.enter_context(tc.tile_pool(name="psA", bufs=2, space="PSUM"))
    psM = ctx.enter_context(tc.tile_pool(name="psM", bufs=3, space="PSUM"))
    psO = ctx.enter_context(tc.tile_pool(name="psO", bufs=3, space="PSUM"))

    # Load-order = HW queue FIFO order. The compute-critical small tensors
    # (x of graph 0, then the weights, then the remaining x) go first on the
    # HWDGE queues, ahead of the 8MB adj flood; masks ride SWDGE (gpsimd),
    # they aren't needed until projection time.
    xts, masks = [], []
    for bb in range(BPC):
        xts.append(xt_pool.tile([IN_DIM, N], FP16, tag="xt", name=f"xt{bb}"))
    we_t = consts.tile([IN_DIM, HID], FP16, tag="we")
    wl_t = []
    bl_t = []
    for i in range(NUM_LAYERS):
        # bias broadcast across partitions: A@(msg+b) == A@msg + rowsum(x)b,
        # so adding b to msg on the PSUM->SBUF copy replaces the rank-1
        # rowsum matmuls exactly.
        wl_t.append(consts.tile([HID, HID], FP16, tag=f"wl{i}", name=f"wl{i}"))
        bl_t.append(consts.tile([128, HID], FP32, tag=f"bl{i}", name=f"bl{i}"))
    ones_t = consts.tile([1, HID], FP16, tag="ones")
    nc.vector.memset(ones_t[:], ONES_VAL)

    # PE clock pre-warm: ~3.4us of dependency-free matmuls during the DMA
    # ramp flips the HAM throttle (1.2 -> 2.4 GHz) before real work arrives.
    warm_t = consts.tile([1, 512], FP16, tag="warm")
    nc.vector.memset(warm_t[:], 0.0)

    def warm_mm(n=1):
        for _ in range(n):
            psw = psA.tile([HID, 512], FP32, tag="psA", name="psw")
            nc.tensor.matmul(psw[:], ones_t[:], warm_t[:], start=True, stop=True)

    warm_mm(0)

    adj_tiles = [
        [adj_pool.tile([128, N], FP16, tag="adj", name=f"adj{bb}_{c}")
         for c in range(NC8)]
        for bb in range(BPC)
    ]

    def load_adj_half(bb, t):
        # two half-loads per chunk, all t=0 halves queued before t=1: the
        # first aggregation n-tile's operands land at half the transfer.
        for c in range(NC8):
            nc.sync.dma_start(
                adj_tiles[bb][c][:, t * 512:(t + 1) * 512],
                adjT[bb, c * 128:(c + 1) * 128, t * 512:(t + 1) * 512])

    # Queue FIFO order == delivery order. Graph 0 must become computable as
    # early as possible: its first aggregation needs the t=0 adj halves plus
    # x/W_embed/Wl0/bl0; everything else follows.
    # Small tensors go over SWDGE (gpsimd): its queue spins up at ~6us while
    # the HW DGE queues are still ramping and flooded with adj bytes — the
    # HWDGE path delivered these ~6us later no matter where they sat in the
    # FIFO. adj keeps the entire HWDGE path.
    wp_t = consts.tile([HID, OUT], FP16, tag="wp")
    bp_t = consts.tile([1, OUT], FP16, tag="bp")
    # embed inputs ride the HWDGE queue head (144KB ahead of the adj flood
    # beats SWDGE's ~1us-per-trigger serialization for the critical pair)
    nc.sync.dma_start(xts[0][:], xT[0])
    nc.sync.dma_start(we_t[:], w_embed[:, :])
    nc.sync.dma_start(wl_t[0][:], wl[0])
    nc.sync.dma_start(bl_t[0][:], blT[0].to_broadcast([128, HID]))
    for bb in range(1, BPC):
        nc.gpsimd.dma_start(xts[bb][:], xT[bb])
    for i in range(1, NUM_LAYERS):
        nc.gpsimd.dma_start(wl_t[i][:], wl[i])
        nc.gpsimd.dma_start(bl_t[i][:], blT[i].to_broadcast([128, HID]))
    nc.gpsimd.dma_start(wp_t[:], w_proj[:, :])
    nc.gpsimd.dma_start(bp_t[:], b_proj[:, :])
    for bb in range(BPC):
        mask_t = mask_pool.tile([128, NC8], FP32, tag="mask", name=f"mask{bb}")
        nc.gpsimd.dma_start(mask_t[:], maskT[bb])
        masks.append(mask_t)
    for bb in range(BPC):
        for t in range(NT):
            load_adj_half(bb, t)
    adjs = adj_tiles

    # Projection of graph b is emitted interleaved between graph b+1's
    # aggregation groups: long runs of tiny proj matmuls otherwise drop PE
    # array activity low enough that the HAM re-throttles the clock at
    # every graph boundary.
    def make_proj_units(bb, h):
        o_big = o_pool.tile([128, NC8, OUT], FP32, tag="o", name=f"o{bb}")

        def unit(c):
            po = psO.tile([128, OUT], FP32, tag="psO", name=f"psO{bb}_{c}")
            nc.tensor.matmul(po[:], h[:, c * 128:(c + 1) * 128], wp_t[:],
                             start=True, stop=False)
            nc.tensor.matmul(po[:], ones_t[:], bp_t[:], start=False, stop=True)
            nc.scalar.activation(o_big[:, c, :], po[:], COPY,
                                 scale=masks[bb][:, c:c + 1])

        units = [lambda c=c: unit(c) for c in range(NC8)]
        out_v = out[bb].rearrange("(c p) j -> p c j", p=128)
        half = NC8 // 2

        def store_half(k):
            sl = slice(0, half) if k == 0 else slice(half, NC8)
            nc.sync.dma_start(out_v[:, sl, :], o_big[:, sl, :])

        return units, store_half

    pending = None  # (units-left, store_half, n-done) of the previous graph

    def emit_pending(k):
        nonlocal pending
        if pending is None:
            return
        units, store_half, done = pending
        take = units[:k]
        for u in take:
            u()
        units = units[k:]
        if done < NC8 // 2 <= done + len(take):
            store_half(0)
        if not units:
            store_half(1)
            pending = None
        else:
            pending = (units, store_half, done + len(take))

    def emit_linear(h, i, bb):
        # msg[n, k] = h @ Wl[i] + bl[i], natural layout chunks of 128 nodes
        msg_t = msg_pool.tile([128, NC8, HID], FP16, tag="msg",
                              name=f"msg{bb}_{i}")
        for c in range(NC8):
            pm = psM.tile([128, HID], FP32, tag="psM")
            nc.tensor.matmul(pm[:], h[:, c * 128:(c + 1) * 128], wl_t[i][:],
                             start=True, stop=True)
            nc.vector.tensor_add(msg_t[:, c, :], pm[:], bl_t[i][:])
        return msg_t

    # Prologue: every graph's embed + layer-0 linear depends only on the
    # small early loads — real PE work that fills the window while graph 0's
    # adj is still streaming in (and keeps the HAM clock warm).
    hs, msgs0 = [], []
    for bb in range(BPC):
        h = h_pool.tile([HID, N], FP16, tag="h", name=f"h0_{bb}")
        for t in range(NT):
            ps = psA.tile([HID, 512], FP32, tag="psA")
            nc.tensor.matmul(ps[:], we_t[:], xts[bb][:, t * 512:(t + 1) * 512],
                             start=True, stop=True)
            nc.scalar.copy(h[:, t * 512:(t + 1) * 512], ps[:])
        hs.append(h)
        msgs0.append(emit_linear(h, 0, bb))

    for bb in range(BPC):
        adj_c = adjs[bb]
        h = hs[bb]

        for i in range(NUM_LAYERS):
            msg_t = msgs0[bb] if i == 0 else emit_linear(h, i, bb)
            # aggT[k, n] = msg.T @ adjT (+ b via msg) ; hT = relu(aggT)
            h2 = h_pool.tile([HID, N], FP16, tag="h")
            last_graph_l2 = bb == BPC - 1 and i == NUM_LAYERS - 1
            if last_graph_l2:
                own_units, own_store_half = make_proj_units(bb, h2)
            for t in range(NT):
                ps = psA.tile([HID, 512], FP32, tag="psA")
                for c in range(NC8):
                    nc.tensor.matmul(ps[:], msg_t[:, c, :],
                                     adj_c[c][:, t * 512:(t + 1) * 512],
                                     start=(c == 0), stop=(c == NC8 - 1))
                nc.scalar.activation(h2[:, t * 512:(t + 1) * 512], ps[:], RELU,
                                     scale=S[i] / S[i + 1])
                if last_graph_l2:
                    # the final graph has no successor to hide its proj in;
                    # each relu half releases its own four proj chunks and
                    # stores them immediately.
                    for u in own_units[t * 4:(t + 1) * 4]:
                        u()
                    own_store_half(t)
                else:
                    emit_pending(2)
            h = h2

        emit_pending(NC8)  # flush whatever didn't fit between agg groups
        if bb < BPC - 1:
            pending = (*make_proj_units(bb, h), 0)
    emit_pending(NC8)


def build_nc():
    # Bacc (not raw Bass): its compile() runs generate_event_semaphores,
    # which splits multi-sem waits down to the 1-wait-per-instruction
    # hardware limit walrus enforces.
    nc = bacc.Bacc("TRN2", debug=False, num_devices=N_CORES, num_swdge_queues=2)
    adjT = nc.dram_tensor("adjT", [BPC, N, N], FP16, kind="ExternalInput").ap()
    xT = nc.dram_tensor("xT", [BPC, IN_DIM, N], FP16, kind="ExternalInput").ap()
    maskT = nc.dram_tensor("maskT", [BPC, 128, NC8], FP32, kind="ExternalInput").ap()
    w_embed = nc.dram_tensor("w_embed", [IN_DIM, HID], FP16, kind="ExternalInput").ap()
    wl = nc.dram_tensor("wl", [NUM_LAYERS, HID, HID], FP16, kind="ExternalInput").ap()
    blT = nc.dram_tensor("blT", [NUM_LAYERS, 1, HID], FP32, kind="ExternalInput").ap()
    w_proj = nc.dram_tensor("w_proj", [HID, OUT], FP16, kind="ExternalInput").ap()
    b_proj = nc.dram_tensor("b_proj", [1, OUT], FP16, kind="ExternalInput").ap()
    out = nc.dram_tensor("out", [BPC, N, OUT], FP32, kind="ExternalOutput").ap()

    with tile.TileContext(nc) as tc, ExitStack() as ctx:
        _kernel_body(ctx, tc, out, adjT, xT, maskT,
                     w_embed, wl, blT, w_proj, b_proj)
    nc.compile()
    return nc


def make_in_maps(node_features, adjacency_matrix, node_mask, W_embed, Wl, bl,
                 W_proj, b_proj):
    x = np.asarray(node_features, dtype=np.float32)
    adj = np.asarray(adjacency_matrix, dtype=np.float32)
    mask = np.asarray(node_mask, dtype=np.float32)
    bl_scaled = np.asarray(bl, dtype=np.float64) / np.array(S[:NUM_LAYERS])[:, None]
    shared = {
        "w_embed": np.asarray(W_embed, dtype=np.float16),
        "wl": np.asarray(Wl, dtype=np.float16),
        "blT": bl_scaled.astype(np.float32).reshape(NUM_LAYERS, 1, HID),
        "w_proj": np.asarray(W_proj, dtype=np.float16),
        "b_proj": (np.asarray(b_proj, np.float64) / (ONES_VAL * S[NUM_LAYERS]))
        .astype(np.float16).reshape(1, OUT),
    }
    in_maps = []
    for c in range(N_CORES):
        sl = slice(c * BPC, (c + 1) * BPC)
        in_maps.append({
            "adjT": np.ascontiguousarray(
                adj[sl].transpose(0, 2, 1)).astype(np.float16),
            "xT": np.ascontiguousarray(x[sl].transpose(0, 2, 1)).astype(np.float16),
            "maskT": np.ascontiguousarray(
                mask[sl].reshape(BPC, NC8, 128).transpose(0, 2, 1))
            * np.float32(S[NUM_LAYERS]),
            **shared,
        })
    return in_maps


_NC_CACHE = None


def get_nc():
    global _NC_CACHE
    if _NC_CACHE is None:
        _NC_CACHE = build_nc()
    return _NC_CACHE


def kernel(**inputs):
    nc = get_nc()
    in_maps = make_in_maps(**inputs)
    res = run_bass_kernel_spmd(nc, in_maps, list(range(N_CORES)))
    outs = [np.asarray(res.results[c]["out"], dtype=np.float32)
            for c in range(N_CORES)]
    return np.concatenate(outs, axis=0)


if __name__ == "__main__":
    rng = np.random.default_rng(0)
    ins = {
        "node_features": rng.standard_normal((B, N, IN_DIM), dtype=np.float32),
        "adjacency_matrix": rng.random((B, N, N), dtype=np.float32),
        "node_mask": np.ones((B, N, 1), np.float32),
        "W_embed": rng.standard_normal((IN_DIM, HID), dtype=np.float32) * 0.1,
        "Wl": rng.standard_normal((NUM_LAYERS, HID, HID), dtype=np.float32) * 0.08,
        "bl": rng.standard_normal((NUM_LAYERS, HID), dtype=np.float32) * 0.08,
        "W_proj": rng.standard_normal((HID, 2 * 32), dtype=np.float32) * 0.08,
        "b_proj": rng.standard_normal((2 * 32,), dtype=np.float32) * 0.08,
    }
    out = kernel(**ins)
    print("out", out.shape, out.dtype, float(np.abs(out).mean()))



# revision 5
# speedup vs baseline: 1.2360x; 1.2360x over previous
"""Bass/Trainium2 kernel for nn_EuclideanGraphEncoder (GCN message passing).

Strategy: data-parallel over the batch (4 graphs per core, 8 cores),
weights replicated, no collectives.

v2 redesign around fp8 DoubleRow aggregation:
  - The adjacency is shipped as fp8e4 (x16 scale) in a pair-interleaved
    layout [128, 4, 2, 1024] so each aggregation matmul contracts 256
    nodes per instruction (MatmulPerfMode.DoubleRow): 4 MMs per 512-col
    PSUM tile instead of 8. Halves both PE time and adj DMA bytes.
  - msg tiles are produced in fp8e4 directly by the DVE bias-add that
    drains the linear-layer PSUM (bias as a pre-broadcast SBUF tile,
    built on-chip at startup by rank-1 matmuls that double as HAM
    clock warm-up).
  - The embedding is folded into layer 0 on the host: W0' = We @ Wl0,
    so the device's first linear runs straight off x (K=64).
  - Aggregation PSUM is [128, 1024] (2 banks) so one scalar ACT does
    relu + rescale for a whole graph-layer.
  - Projection: 8 chunk matmuls + one rank-1 bias matmul accumulate in
    a single PSUM bank; per-chunk ACT applies the node mask and writes
    fp16 output (host multiplies by SO and upcasts).
  - DMA: adjacency rides the SP HWDGE ring as ONE 1MB contiguous-per-
    partition dma_start per graph (8KB bursts); x/weights/bias-rows ride
    the ACT HWDGE ring in parallel; masks ride SWDGE. This keeps the
    per-trigger (~0.7us) serialization off the critical path.

Scales (exact powers of two, folded into host-side weights):
  adj8 = fp8(16*adj); msg_dev = msg_true/Sm[i]; h_dev = h_true/Sh[i];
  out = fp16((h3@Wp + bp)/So) * mask;  host returns out*So as f32.
"""

import sys
from contextlib import ExitStack

import numpy as np
import ml_dtypes

try:
    import concourse.bass as bass
except ImportError:  # fall back to the repo checkout
    sys.path.insert(0, "/opt/trn_rl_repo")
    import concourse.bass as bass

import concourse.tile as tile
from concourse import bacc, mybir
from concourse.bass_utils import run_bass_kernel_spmd

B, N, IN_DIM, HID, OUT = 32, 1024, 64, 128, 64
NUM_LAYERS = 3
N_CORES = 8
BPC = B // N_CORES  # graphs per core
NC8 = N // 128      # node chunks of 128
NPAIR = NC8 // 2    # DoubleRow chunk pairs (256 nodes each)

FP8 = mybir.dt.float8e4
FP16 = mybir.dt.float16
FP32 = mybir.dt.float32
RELU = mybir.ActivationFunctionType.Relu
COPY = mybir.ActivationFunctionType.Copy
DR = mybir.MatmulPerfMode.DoubleRow

# numeric scales (see module docstring); all exact powers of two
ADJ_SCALE = 16.0
SM = [2.0 ** -5, 2.0 ** -1, 2.0 ** 7]        # msg_dev = msg_true / SM[i]
SH = [None, 2.0 ** -7, 2.0, 2.0 ** 8]        # h_dev = h_true / SH[i]
SO = 2.0 ** 7                                 # out_dev = out_true / SO
RELU_SCALE = [SM[i] / (ADJ_SCALE * SH[i + 1]) for i in range(3)]


def _kernel_body(ctx, tc, out, adj8, xT, maskT, wpack, rows):
    nc = tc.nc

    consts = ctx.enter_context(tc.tile_pool(name="consts", bufs=1))
    adj_pool = ctx.enter_context(tc.tile_pool(name="adj", bufs=BPC))
    xt_pool = ctx.enter_context(tc.tile_pool(name="xt", bufs=BPC))
    mask_pool = ctx.enter_context(tc.tile_pool(name="mask", bufs=BPC))
    h_pool = ctx.enter_context(tc.tile_pool(name="h", bufs=6))
    msg_pool = ctx.enter_context(tc.tile_pool(name="msg", bufs=4))
    o_pool = ctx.enter_context(tc.tile_pool(name="o", bufs=2))
    bl_pool = ctx.enter_context(tc.tile_pool(name="bl", bufs=NUM_LAYERS))
    psA = ctx.enter_context(tc.tile_pool(name="psA", bufs=2, space="PSUM"))
    psM = ctx.enter_context(tc.tile_pool(name="psM", bufs=2, space="PSUM"))
    psO = ctx.enter_context(tc.tile_pool(name="psO", bufs=2, space="PSUM"))

    ones_t = consts.tile([1, HID], FP16, tag="ones")
    nc.vector.memset(ones_t[:], 1.0)

    wpack_t = consts.tile([128, 448], FP16, tag="wpack")
    rows_t = consts.tile([1, 2048], FP16, tag="rows")
    w0_ap = wpack_t[0:IN_DIM, 0:HID]
    wl_ap = [None, wpack_t[:, 128:256], wpack_t[:, 256:384]]
    wp_ap = wpack_t[:, 384:448]
    bp_row = rows_t[:, 1536:2048]

    # ACT HWDGE ring: x0 + weights + bias rows first, then x1-3.
    xts = []
    for bb in range(BPC):
        xts.append(xt_pool.tile([IN_DIM, N], FP16, tag="xt", name=f"xt{bb}"))
    nc.scalar.dma_start(xts[0][:], xT[0])
    nc.scalar.dma_start(wpack_t[:], wpack[:, :])
    nc.scalar.dma_start(rows_t[:], rows[:, :])
    for bb in range(1, BPC):
        nc.scalar.dma_start(xts[bb][:], xT[bb])

    # SP HWDGE ring: the adjacency flood, one 1MB DMA per graph
    # (contiguous 8KB per partition), then (later) the output stores.
    adjs = []
    for bb in range(BPC):
        a = adj_pool.tile([128, NPAIR, 2, N], FP8, tag="adj", name=f"adj{bb}")
        nc.sync.dma_start(a[:], adj8[bb])
        adjs.append(a)

    # masks ride SWDGE (gpsimd): tiny, needed only from projection time.
    masks = []
    for bb in range(BPC):
        m = mask_pool.tile([128, NC8], FP32, tag="mask", name=f"mask{bb}")
        nc.gpsimd.dma_start(m[:], maskT[bb])
        masks.append(m)

    # HAM warm-up: dependency-free rank-1 matmuls keep the PE busy window
    # open from ~6.5us so the 2.4GHz un-throttle lands before real work.
    warm_ps = psO.tile([128, 512], FP32, tag="psO", name="warm")
    for _ in range(8):
        nc.tensor.matmul(warm_ps[:, 0:HID], ones_t[:], ones_t[:],
                         start=True, stop=True)

    # Bias broadcast tiles, built on-chip: rank-1 (ones x row) -> PSUM,
    # DVE copy -> SBUF fp16 [128, 512] (bias pattern tiled x4 chunks).
    bl_bcast = []
    for i in range(NUM_LAYERS):
        ps = psM.tile([128, 512], FP32, tag="psM", name=f"blps{i}")
        nc.tensor.matmul(ps[:], ones_t[:], rows_t[:, i * 512:(i + 1) * 512],
                         start=True, stop=True)
        bt = bl_pool.tile([128, 512], FP16, tag="bl", name=f"bl{i}")
        nc.vector.tensor_copy(bt[:], ps[:])
        bl_bcast.append(bt)

    def emit_linear(bb, i, h):
        # msg[n, k] = (h @ Wl'[i] + bl'[i]) -> fp8 pair layout
        msg_t = msg_pool.tile([128, NPAIR, 2, HID], FP8, tag="msg",
                              name=f"msg{bb}_{i}")
        for half in range(2):
            pm = psM.tile([128, 512], FP32, tag="psM")
            for k in range(4):
                c = 4 * half + k
                if i == 0:
                    lhsT = xts[bb][:, c * 128:(c + 1) * 128]
                    rhs = w0_ap
                else:
                    lhsT = h[:, c * 128:(c + 1) * 128]
                    rhs = wl_ap[i]
                nc.tensor.matmul(pm[:, k * 128:(k + 1) * 128], lhsT, rhs,
                                 start=True, stop=True)
            half_ap = msg_t[:, 2 * half:2 * half + 2, :, :]
            nc.vector.tensor_add(
                half_ap.rearrange("p a b c -> p (a b c)"), pm[:],
                bl_bcast[i][:])
        return msg_t

    # Projection: 8 chunk MMs + rank-1 bias accumulate in one PSUM bank;
    # the per-chunk mask ACTs + stores are deferred (interleaved into the
    # next graph's aggregation) via the pending machinery below.
    def make_proj_units(bb, h):
        o_big = o_pool.tile([128, NC8, OUT], FP16, tag="o", name=f"o{bb}")
        po = psO.tile([128, 512], FP32, tag="psO", name=f"psO{bb}")
        for c in range(NC8):
            nc.tensor.matmul(po[:, c * OUT:(c + 1) * OUT],
                             h[:, c * 128:(c + 1) * 128], wp_ap,
                             start=(c == 0), stop=False, skip_group_check=True)
        nc.tensor.matmul(po[:], ones_t[:], bp_row, start=False, stop=True,
                         skip_group_check=True)

        def unit(c):
            nc.scalar.activation(o_big[:, c, :], po[:, c * OUT:(c + 1) * OUT],
                                 COPY, scale=masks[bb][:, c:c + 1])

        units = [lambda c=c: unit(c) for c in range(NC8)]
        out_v = out[bb].rearrange("(c p) j -> p c j", p=128)
        half = NC8 // 2

        def store_half(k):
            sl = slice(0, half) if k == 0 else slice(half, NC8)
            nc.sync.dma_start(out_v[:, sl, :], o_big[:, sl, :])

        return units, store_half

    pending = None  # (units-left, store_half, n-done) of the previous graph

    def emit_pending(k):
        nonlocal pending
        if pending is None:
            return
        units, store_half, done = pending
        take = units[:k]
        for u in take:
            u()
        units = units[k:]
        if done < NC8 // 2 <= done + len(take):
            store_half(0)
        if not units:
            store_half(1)
            pending = None
        else:
            pending = (units, store_half, done + len(take))

    # Prologue: every graph's layer-0 linear depends only on x + wpack —
    # real PE work while the adjacency is still streaming in.
    msgs0 = [emit_linear(bb, 0, None) for bb in range(BPC)]

    for bb in range(BPC):
        adj_t = adjs[bb]
        h = None
        for i in range(NUM_LAYERS):
            msg_t = msgs0[bb] if i == 0 else emit_linear(bb, i, h)
            pa = psA.tile([128, N], FP32, tag="psA")
            last = bb == BPC - 1 and i == NUM_LAYERS - 1
            for t in range(2):
                for c2 in range(NPAIR):
                    nc.tensor.matmul(
                        pa[:, t * 512:(t + 1) * 512],
                        msg_t[:, c2, :, :],
                        adj_t[:, c2, :, t * 512:(t + 1) * 512],
                        start=(c2 == 0), stop=(c2 == NPAIR - 1), perf_mode=DR)
                emit_pending(2 if not last else 0)
            h2 = h_pool.tile([HID, N], FP16, tag="h")
            nc.scalar.activation(h2[:], pa[:], RELU, scale=RELU_SCALE[i])
            emit_pending(2 if not last else 0)
            h = h2

        emit_pending(NC8)  # flush anything left of the previous graph
        units, store_half = make_proj_units(bb, h)
        if bb < BPC - 1:
            pending = (units, store_half, 0)
        else:
            for u in units:
                u()
            store_half(0)
            store_half(1)


def build_nc():
    # Bacc (not raw Bass): its compile() runs generate_event_semaphores,
    # which splits multi-sem waits down to the 1-wait-per-instruction
    # hardware limit walrus enforces.
    nc = bacc.Bacc("TRN2", debug=False, num_devices=N_CORES, num_swdge_queues=2)
    adj8 = nc.dram_tensor("adj8", [BPC, 128, NPAIR, 2, N], FP8,
                          kind="ExternalInput").ap()
    xT = nc.dram_tensor("xT", [BPC, IN_DIM, N], FP16, kind="ExternalInput").ap()
    maskT = nc.dram_tensor("maskT", [BPC, 128, NC8], FP32, kind="ExternalInput").ap()
    wpack = nc.dram_tensor("wpack", [128, 448], FP16, kind="ExternalInput").ap()
    rows = nc.dram_tensor("rows", [1, 2048], FP16, kind="ExternalInput").ap()
    out = nc.dram_tensor("out", [BPC, N, OUT], FP16, kind="ExternalOutput").ap()

    with tile.TileContext(nc) as tc, ExitStack() as ctx:
        _kernel_body(ctx, tc, out, adj8, xT, maskT, wpack, rows)
    nc.compile()
    return nc


def make_in_maps(node_features, adjacency_matrix, node_mask, W_embed, Wl, bl,
                 W_proj, b_proj):
    e4 = ml_dtypes.float8_e4m3
    x = np.asarray(node_features, dtype=np.float32)
    adj = np.asarray(adjacency_matrix, dtype=np.float32)
    mask = np.asarray(node_mask, dtype=np.float32)
    We = np.asarray(W_embed, np.float64)
    Wl64 = np.asarray(Wl, np.float64)
    bl64 = np.asarray(bl, np.float64)
    Wp = np.asarray(W_proj, np.float64)
    bp = np.asarray(b_proj, np.float64)

    wpack = np.zeros((128, 448), np.float16)
    wpack[:IN_DIM, 0:128] = (We @ Wl64[0] / SM[0]).astype(np.float16)
    wpack[:, 128:256] = (Wl64[1] * (SH[1] / SM[1])).astype(np.float16)
    wpack[:, 256:384] = (Wl64[2] * (SH[2] / SM[2])).astype(np.float16)
    wpack[:, 384:448] = (Wp * (SH[3] / SO)).astype(np.float16)

    rows = np.concatenate(
        [np.tile(bl64[i] / SM[i], 4) for i in range(NUM_LAYERS)]
        + [np.tile(bp / SO, NC8)]).astype(np.float16).reshape(1, 2048)

    in_maps = []
    for cc in range(N_CORES):
        sl = slice(cc * BPC, (cc + 1) * BPC)
        # adj8[bb, j, c2, o, n] = fp8(16 * adj[n, c2*256 + o*128 + j])
        a = np.ascontiguousarray(adj[sl].transpose(0, 2, 1))  # [BPC, m, n]
        a = a.reshape(BPC, NPAIR, 2, 128, N).transpose(0, 3, 1, 2, 4)
        in_maps.append({
            "adj8": (np.float32(ADJ_SCALE) * a).astype(e4),
            "xT": np.ascontiguousarray(x[sl].transpose(0, 2, 1)).astype(np.float16),
            "maskT": np.ascontiguousarray(
                mask[sl].reshape(BPC, NC8, 128).transpose(0, 2, 1)),
            "wpack": wpack,
            "rows": rows,
        })
    return in_maps


_NC_CACHE = None


def get_nc():
    global _NC_CACHE
    if _NC_CACHE is None:
        _NC_CACHE = build_nc()
    return _NC_CACHE


def postprocess(raw_out):
    return np.asarray(raw_out, np.float32) * np.float32(SO)


def kernel(**inputs):
    nc = get_nc()
    in_maps = make_in_maps(**inputs)
    res = run_bass_kernel_spmd(nc, in_maps, list(range(N_CORES)))
    outs = [postprocess(res.results[c]["out"]) for c in range(N_CORES)]
    return np.concatenate(outs, axis=0)


if __name__ == "__main__":
    rng = np.random.default_rng(0)
    ins = {
        "node_features": rng.standard_normal((B, N, IN_DIM), dtype=np.float32),
        "adjacency_matrix": rng.random((B, N, N), dtype=np.float32),
        "node_mask": np.ones((B, N, 1), np.float32),
        "W_embed": rng.standard_normal((IN_DIM, HID), dtype=np.float32) * 0.1,
        "Wl": rng.standard_normal((NUM_LAYERS, HID, HID), dtype=np.float32) * 0.08,
        "bl": rng.standard_normal((NUM_LAYERS, HID), dtype=np.float32) * 0.08,
        "W_proj": rng.standard_normal((HID, 2 * 32), dtype=np.float32) * 0.08,
        "b_proj": rng.standard_normal((2 * 32,), dtype=np.float32) * 0.08,
    }
    out = kernel(**ins)
    print("out", out.shape, out.dtype, float(np.abs(out).mean()))
